# revision 28
# baseline (speedup 1.0000x reference)
"""Multi-head attention forward (B=2, S=2048, D=1024, H=16) on 8 TRN2 cores.

Sharding: hybrid tensor/data parallel. Cores 0-3 take batch 0, cores 4-7
batch 1; within a batch each core owns 4 heads (256 of 1024 features).
The host pre-transposes activations/weights so the device kernel needs no
on-device transposes, and sums the 4 partial output projections per batch
(+ output bias) at the end.

Per-core dataflow (v4):
  qT/kT = W @ X.T            (PE, feature-on-partition; bias added by DVE
                              tensor_scalar during the PSUM->SBUF move)
  V     = X @ Wv.T           (PE, token-on-partition "natural" layout --
                              no transposes; bias via host-replicated tile)
  sT    = kT.T @ qT          (PE; the two heads of a pair use row groups
                              0-63 / 64-127 and co-stream)
  eT    = exp(sT)            (ACT; scores ~ N(0,1), no max-subtract)
  ctxT  = v_aug.T @ eT       (PE split-K: keys 0-63 / 64-127 as row-tiles
                              into separate PSUM banks, co-streamed; 65th
                              v column accumulates softmax denominators)
  norm:  cxs = cxa+cxb (DVE), reciprocal_approx_fast, GPSIMD partition-
         broadcast of 1/denom, DVE mul -> ctx (fp16). No PE, no PSUM.
  out   = ctxT.T @ WoT       (PE, accumulated over head pairs)

Scheduling: the attention k-loop is ACT(exp)-bound (~1.0us/kt). The score
PSUM ring (tag "big") carries ONLY score tiles mid-loop so its double
buffering is never phase-slipped; output-projection units are batched
right after kt0/kt1 of the next k-loop (their matmuls overlap the first
exps), and PV ramps in 4 iterations late at each pair start so the
previous pair's norm (DVE-only) can release the cx PSUM banks without
stalling the PE.
"""

import sys
import types
from collections import deque

import numpy as np

# ---------------------------------------------------------------------------
# Problem constants (hardcoded; kernel.py must be self-contained)
# ---------------------------------------------------------------------------
B = 2  # batch
S = 2048  # sequence length
D = 1024  # model dim
H = 16  # heads
DK = D // H  # 64 head dim
NCORES = 8
CPB = NCORES // B  # cores per batch = 4
FH = D // CPB  # features per core = 256 (4 heads)
P = 128
KD = D // P  # 8 contraction k-tiles for projections
KT = S // P  # 16 key-token tiles
NM = FH // P  # 2 m-tiles per core = head pairs
QS = 512  # q-slice width for the attention inner loop
NQS = S // QS  # 4
VW = 66  # padded per-head width of the augmented V tile (DK + 1 + pad)
NEG_SCALE = 1.0 / np.sqrt(DK)  # folded into Wq/bq on the host


def _install_ntff_hook():
    """Recreate antenv.axon_hooks so trace=True can profile via axon."""
    if "antenv.axon_hooks" in sys.modules:
        return
    try:
        import antenv
    except ImportError:
        return
    mod = types.ModuleType("antenv.axon_hooks")
    mod._hook = None
    mod.set_axon_ntff_profile_hook = lambda h: setattr(mod, "_hook", h)
    mod.get_axon_ntff_profile_hook = lambda: mod._hook
    sys.modules["antenv.axon_hooks"] = mod
    antenv.axon_hooks = mod
    try:
        from trn_agent_boot.trn_boot import _ntff_profile_via_ctypes

        mod.set_axon_ntff_profile_hook(
            _ntff_profile_via_ctypes("/opt/axon/libaxon_pjrt.so")
        )
    except Exception:
        pass


_NC_CACHE = {}


def _build_nc():
    """Build the per-core Bass program (identical on all 8 cores)."""
    from contextlib import ExitStack

    import concourse.bass as bass  # noqa: F401
    import concourse.mybir as mybir
    import concourse.tile as tile
    from concourse import bacc

    f32 = mybir.dt.float32
    f16 = mybir.dt.float16
    AF = mybir.ActivationFunctionType

    nc = bacc.Bacc()

    xtq = nc.dram_tensor("xtq", [D, S], f16, kind="ExternalInput")
    xtk = nc.dram_tensor("xtk", [D, S], f16, kind="ExternalInput")
    xtv = nc.dram_tensor("xtv", [D, S], f16, kind="ExternalInput")
    wqt = nc.dram_tensor("wqt", [D, FH], f16, kind="ExternalInput")
    wkt = nc.dram_tensor("wkt", [D, FH], f16, kind="ExternalInput")
    wvt = nc.dram_tensor("wvt", [D, FH], f16, kind="ExternalInput")
    wot = nc.dram_tensor("wot", [FH, D], f16, kind="ExternalInput")
    bqd = nc.dram_tensor("bqd", [P, NM], f32, kind="ExternalInput")
    bkd = nc.dram_tensor("bkd", [P, NM], f32, kind="ExternalInput")
    bvr = nc.dram_tensor("bvr", [P, FH], f32, kind="ExternalInput")
    out = nc.dram_tensor("out", [S, D], f16, kind="ExternalOutput")

    with tile.TileContext(nc) as tc, ExitStack() as ctx:
        const = ctx.enter_context(tc.tile_pool(name="const", bufs=1))
        wpool = ctx.enter_context(tc.tile_pool(name="wpool", bufs=1))
        persist = ctx.enter_context(tc.tile_pool(name="persist", bufs=1))
        xpool = ctx.enter_context(tc.tile_pool(name="xpool", bufs=3))
        expool = ctx.enter_context(tc.tile_pool(name="expool", bufs=6))
        npool = ctx.enter_context(tc.tile_pool(name="npool", bufs=2))
        obpool = ctx.enter_context(tc.tile_pool(name="obpool", bufs=3))

        # --- constants ---
        bq_sb = const.tile([P, NM], f32)
        bk_sb = const.tile([P, NM], f32)
        bvr_sb = const.tile([P, FH], f32)
        nc.sync.dma_start(bq_sb, bqd[:, :])
        nc.sync.dma_start(bk_sb, bkd[:, :])
        nc.sync.dma_start(bvr_sb, bvr[:, :])

        # --- weight tiles ---
        wq_sb = wpool.tile([P, KD, FH], f16)
        wk_sb = wpool.tile([P, KD, FH], f16)
        wv_sb = wpool.tile([P, KD, FH], f16)
        wo_sb = wpool.tile([P, NM, D], f16)

        # --- persistent activations ---
        qt_sb = persist.tile([P, NM, S], f16)
        kt_sb = persist.tile([P, NM, S], f16)
        vaug_sb = persist.tile([P, KT, CPB, VW], f16)
        xv_sb = persist.tile([P, KD, S], f16)
        ctx_sb = persist.tile([P, NM, S], f16)

        # xtv streams on the scalar engine's DMA queue from t=0 so it is
        # resident by the time the V projection starts, in parallel with
        # the q/k input stream on the sync queue.
        for ko in range(KD):
            nc.scalar.dma_start(xv_sb[:, ko, :], xtv[ko * P : (ko + 1) * P, :])

        nc.vector.memset(vaug_sb, 0.0)
        nc.vector.memset(vaug_sb[:, :, :, DK : DK + 1], 1.0)

        # ------------------------------------------------------------------
        # Phase 1a: qT/kT = W @ X.T, bias via DVE tensor_scalar on the move
        # ------------------------------------------------------------------
        with tc.tile_pool(name="pp", bufs=2, space="PSUM") as pp:
            for xdram, wdram, w_sb, b_sb, dst in (
                (xtq, wqt, wq_sb, bq_sb, qt_sb),
                (xtk, wkt, wk_sb, bk_sb, kt_sb),
            ):
                ps = [pp.tile([P, S], f32, tag="pp", name=f"ps{m}") for m in range(NM)]
                for ko in range(KD):
                    nc.sync.dma_start(
                        w_sb[:, ko, :], wdram[ko * P : (ko + 1) * P, :]
                    )
                    # split each x tile across two DMA queues for bandwidth
                    # (gpsimd's queue wakes late -- keep ko=0 fully on sync)
                    xt_t = xpool.tile([P, S], f16, tag="xt")
                    nc.sync.dma_start(
                        xt_t[:, 0 : S // 2], xdram[ko * P : (ko + 1) * P, 0 : S // 2]
                    )
                    eng = nc.sync if (xdram is xtq and ko == 0) else nc.gpsimd
                    eng.dma_start(
                        xt_t[:, S // 2 : S], xdram[ko * P : (ko + 1) * P, S // 2 : S]
                    )
                    for m in range(NM):
                        for ns in range(S // 512):
                            nc.tensor.matmul(
                                ps[m][:, ns * 512 : (ns + 1) * 512],
                                lhsT=w_sb[:, ko, m * P : (m + 1) * P],
                                rhs=xt_t[:, ns * 512 : (ns + 1) * 512],
                                start=(ko == 0),
                                stop=(ko == KD - 1),
                            )
                for m in range(NM):
                    nc.vector.tensor_scalar_add(
                        dst[:, m, :], ps[m][:, :], b_sb[:, m : m + 1]
                    )

        # ------------------------------------------------------------------
        # Phase 1b: V natural = X @ Wv.T per 128-token chunk (no transposes)
        # ------------------------------------------------------------------
        bvr4 = bvr_sb.rearrange("p (h x) -> p h x", x=DK)
        vaug4 = vaug_sb  # [P, kt, head, VW]
        for ko in range(KD):
            nc.sync.dma_start(wv_sb[:, ko, :], wvt[ko * P : (ko + 1) * P, :])
        nc.sync.dma_start(wo_sb, wot[:, :].rearrange("(m p) d -> p m d", p=P))
        with tc.tile_pool(name="vp", bufs=2, space="PSUM") as vp:
            for tcn in range(KT):
                pvn = vp.tile([P, FH], f32, tag="vn", name=f"pvn{tcn}")
                for ko in range(KD):
                    nc.tensor.matmul(
                        pvn,
                        lhsT=xv_sb[:, ko, tcn * P : (tcn + 1) * P],
                        rhs=wv_sb[:, ko, :],
                        start=(ko == 0),
                        stop=(ko == KD - 1),
                    )
                nc.vector.tensor_add(
                    vaug4[:, tcn, :, 0:DK],
                    bvr4,
                    pvn.rearrange("p (h x) -> p h x", x=DK),
                )

        # ------------------------------------------------------------------
        # Phase 2: attention + deferred norm / output projection
        # ------------------------------------------------------------------
        with (
            tc.tile_pool(name="big", bufs=2, space="PSUM") as big,
            tc.tile_pool(name="pcxa", bufs=2, space="PSUM") as pcxa,
            tc.tile_pool(name="pcxb", bufs=2, space="PSUM") as pcxb,
        ):
            pending = deque()  # DVE-only closures (norm muls)
            op_queue = deque()  # output-projection units (PE+DVE+DMA)

            def drain(n=1):
                for _ in range(n):
                    if pending:
                        pending.popleft()()

            def out_proj(mt, ns, pool, tag):
                # psum comes from a cx ring (freed by the boundary merge) so
                # the score ring's double-buffering is never disturbed
                ops = pool.tile([P, 512], f32, tag=tag, name=f"op{mt}_{ns}")
                for pair in range(NM):
                    nc.tensor.matmul(
                        ops,
                        lhsT=ctx_sb[:, pair, mt * P : (mt + 1) * P],
                        rhs=wo_sb[:, pair, ns * 512 : (ns + 1) * 512],
                        start=(pair == 0),
                        stop=(pair == NM - 1),
                    )
                ob = obpool.tile([P, 512], f16, tag="ob", name=f"ob{mt}_{ns}")
                nc.vector.tensor_copy(ob, ops)
                nc.sync.dma_start(
                    out[mt * P : (mt + 1) * P, ns * 512 : (ns + 1) * 512], ob
                )

            for qs in range(NQS):
                q0 = qs * QS
                for pair in range(NM):
                    cx = {}  # allocated lazily at the first PV emission

                    def pv(pv_kt, pv_ex, cx=cx, pair=pair, qs=qs):
                        if not cx:
                            cx["a"] = [
                                pcxa.tile(
                                    [DK + 1, QS],
                                    f32,
                                    tag="cxa",
                                    name=f"cxa{pair}_{qs}_{h}",
                                )
                                for h in range(2)
                            ]
                            cx["b"] = [
                                pcxb.tile(
                                    [DK + 1, QS],
                                    f32,
                                    tag="cxb",
                                    name=f"cxb{pair}_{qs}_{h}",
                                )
                                for h in range(2)
                            ]
                        cxa, cxb = cx["a"], cx["b"]
                        for h in range(2):
                            nc.tensor.matmul(
                                cxa[h],
                                lhsT=vaug4[0:DK, pv_kt, 2 * pair + h, 0 : DK + 1],
                                rhs=pv_ex[0:DK, h * QS : (h + 1) * QS],
                                start=(pv_kt == 0),
                                stop=(pv_kt == KT - 1),
                            )
                            nc.tensor.matmul(
                                cxb[h],
                                lhsT=vaug4[DK:P, pv_kt, 2 * pair + h, 0 : DK + 1],
                                rhs=pv_ex[DK:P, h * QS : (h + 1) * QS],
                                start=(pv_kt == 0),
                                stop=(pv_kt == KT - 1),
                            )

                    exq = deque()
                    for kt in range(KT):
                        sc = big.tile(
                            [P, 2 * QS], f32, tag="big", name=f"sc{pair}_{qs}_{kt}"
                        )
                        for h in range(2):
                            nc.tensor.matmul(
                                sc[:, h * QS : (h + 1) * QS],
                                lhsT=kt_sb[
                                    DK * h : DK * (h + 1),
                                    pair,
                                    kt * P : (kt + 1) * P,
                                ],
                                rhs=qt_sb[
                                    DK * h : DK * (h + 1), pair, q0 : q0 + QS
                                ],
                                start=True,
                                stop=True,
                            )
                        ex = expool.tile([P, 2 * QS], f16, tag="ex")
                        nc.scalar.activation(ex, sc, AF.Exp)
                        exq.append((kt, ex))
                        if kt == 1:
                            # out-proj batch a: psum from the cxa ring (freed
                            # by the boundary merge); overlaps exp(kt0/kt1)
                            for _ in range(min(2, len(op_queue))):
                                op_queue.popleft()(pcxa, "cxa")
                        elif kt == 2:
                            for _ in range(min(2, len(op_queue))):
                                op_queue.popleft()(pcxb, "cxb")
                        # PV ramps in late (cx banks release) then catches up
                        emitted = 0
                        lim = 0 if kt < 4 else 2
                        while len(exq) > 2 and emitted < lim:
                            pv(*exq.popleft())
                            emitted += 1
                        if kt >= 5:
                            drain(1)
                    while exq:
                        pv(*exq.popleft())
                    cxa, cxb = cx["a"], cx["b"]

                    # --- inline norm prep (DVE/DMA/GPSIMD only, no PE) ---
                    cxs = [
                        npool.tile(
                            [DK + 1, QS],
                            f32,
                            tag="cxs",
                            bufs=4,
                            name=f"cxs{pair}_{qs}_{h}",
                        )
                        for h in range(2)
                    ]
                    for h in range(2):
                        nc.vector.tensor_copy(cxs[h], cxa[h])
                        nc.vector.tensor_add(cxs[h], cxs[h], cxb[h])
                    s2 = npool.tile([2, QS], f32, tag="s2", name=f"s2_{pair}_{qs}")
                    nc.sync.dma_start(s2[0:1, :], cxs[0][DK : DK + 1, :])
                    nc.sync.dma_start(s2[1:2, :], cxs[1][DK : DK + 1, :])
                    rc = npool.tile([2, QS], f32, tag="rc", name=f"rc{pair}_{qs}")
                    nc.vector.reciprocal_approx_fast(rc, s2)
                    rcf = npool.tile([2, QS], f16, tag="rcf", name=f"rcf{pair}_{qs}")
                    nc.vector.tensor_copy(rcf, rc)
                    rcB = npool.tile([1, QS], f16, tag="rcB", name=f"rcB{pair}_{qs}")
                    nc.sync.dma_start(rcB, rcf[1:2, :])
                    bcs = [
                        npool.tile(
                            [DK, QS], f16, tag="bcs", bufs=4, name=f"bc{pair}_{qs}_{h}"
                        )
                        for h in range(2)
                    ]
                    nc.gpsimd.partition_broadcast(bcs[0], rcf[0:1, :])
                    nc.gpsimd.partition_broadcast(bcs[1], rcB)

                    def mul_h(p, q, h, cxs_h, bc_h):
                        nc.vector.tensor_mul(
                            ctx_sb[DK * h : DK * (h + 1), p, q * QS : (q + 1) * QS],
                            cxs_h[0:DK, :],
                            bc_h,
                        )

                    for h in range(2):
                        pending.append(
                            lambda p=pair, q=qs, hh=h, a=cxs[h], b=bcs[h]: mul_h(
                                p, q, hh, a, b
                            )
                        )

                # enqueue this qs's output projections only once both pairs'
                # norm muls have been EMITTED (the enqueue rides the pending
                # queue, which drains strictly after the muls) so out_proj
                # never reads ctx_sb ahead of its writers in program order.
                def enq_ops(q=qs):
                    for sub in range(QS // P):
                        for ns in range(D // 512):
                            op_queue.append(
                                lambda pool, tag, m=q * (QS // P) + sub, n=ns: out_proj(
                                    m, n, pool, tag
                                )
                            )

                pending.append(enq_ops)

                # tail of the qs: nothing; leftovers drain at the very end
            # final drain: leftover op units fill the PE while the last
            # pair's norm chain (DVE/GPSIMD) computes; then its own ops
            for j in range(min(4, len(op_queue))):
                if j % 2 == 0:
                    op_queue.popleft()(pcxa, "cxa")
                else:
                    op_queue.popleft()(pcxb, "cxb")
            while pending:
                drain(1)
            alt = 0
            while op_queue:
                if alt % 2 == 0:
                    op_queue.popleft()(pcxa, "cxa")
                else:
                    op_queue.popleft()(pcxb, "cxb")
                alt += 1

    nc.finalize()
    return nc


def _get_nc():
    if "nc" not in _NC_CACHE:
        _install_ntff_hook()
        _NC_CACHE["nc"] = _build_nc()
    return _NC_CACHE["nc"]


def _make_in_maps(query, key, value, Wq, bq, Wk, bk, Wv, bv, Wo):
    qn = np.asarray(query, np.float32)
    kn = np.asarray(key, np.float32)
    vn = np.asarray(value, np.float32)
    Wq = np.asarray(Wq, np.float32)
    Wk = np.asarray(Wk, np.float32)
    Wv = np.asarray(Wv, np.float32)
    Wo = np.asarray(Wo, np.float32)
    bq = np.asarray(bq, np.float32)
    bk = np.asarray(bk, np.float32)
    bv = np.asarray(bv, np.float32)

    xt = {}
    for b in range(B):
        xt[b] = (
            np.ascontiguousarray(qn[b].T).astype(np.float16),
            np.ascontiguousarray(kn[b].T).astype(np.float16),
            np.ascontiguousarray(vn[b].T).astype(np.float16),
        )

    in_maps = []
    for c in range(NCORES):
        b, hp = divmod(c, CPB)
        sl = slice(hp * FH, (hp + 1) * FH)
        in_maps.append(
            {
                "xtq": xt[b][0],
                "xtk": xt[b][1],
                "xtv": xt[b][2],
                "wqt": np.ascontiguousarray((Wq[sl, :] * NEG_SCALE).T).astype(
                    np.float16
                ),
                "wkt": np.ascontiguousarray(Wk[sl, :].T).astype(np.float16),
                "wvt": np.ascontiguousarray(Wv[sl, :].T).astype(np.float16),
                "wot": np.ascontiguousarray(Wo[:, sl].T).astype(np.float16),
                "bqd": np.ascontiguousarray(
                    (bq[sl] * NEG_SCALE).reshape(NM, P).T
                ),
                "bkd": np.ascontiguousarray(bk[sl].reshape(NM, P).T),
                "bvr": np.ascontiguousarray(np.tile(bv[sl], (P, 1))),
            }
        )
    return in_maps


def _run(inputs, trace=False):
    from concourse.bass_utils import run_bass_kernel_spmd

    nc = _get_nc()
    in_maps = _make_in_maps(
        inputs["query"],
        inputs["key"],
        inputs["value"],
        inputs["Wq"],
        inputs["bq"],
        inputs["Wk"],
        inputs["bk"],
        inputs["Wv"],
        inputs["bv"],
        inputs["Wo"],
    )
    res = run_bass_kernel_spmd(nc, in_maps, list(range(NCORES)), trace=trace)
    bo = np.asarray(inputs["bo"], np.float32)
    out = np.zeros((B, S, D), np.float32)
    for c in range(NCORES):
        out[c // CPB] += res.results[c]["out"].astype(np.float32)
    out += bo[None, None, :]
    return out, res


def kernel(**inputs) -> np.ndarray:
    out, _ = _run(inputs, trace=False)
    return out


# revision 32
# speedup vs baseline: 1.0218x; 1.0218x over previous
"""Multi-head attention forward (B=2, S=2048, D=1024, H=16) on 8 TRN2 cores.

Sharding: hybrid tensor/data parallel. Cores 0-3 take batch 0, cores 4-7
batch 1; within a batch each core owns 4 heads (256 of 1024 features).
The host pre-transposes activations/weights so the device kernel needs no
on-device transposes, and sums the 4 partial output projections per batch
(+ output bias) at the end.

Per-core dataflow (v4):
  qT/kT = W @ X.T            (PE, feature-on-partition; bias added by DVE
                              tensor_scalar during the PSUM->SBUF move)
  V     = X @ Wv.T           (PE, token-on-partition "natural" layout --
                              no transposes; bias via host-replicated tile)
  sT    = kT.T @ qT          (PE; the two heads of a pair use row groups
                              0-63 / 64-127 and co-stream)
  eT    = exp(sT)            (ACT; scores ~ N(0,1), no max-subtract)
  ctxT  = v_aug.T @ eT       (PE split-K: keys 0-63 / 64-127 as row-tiles
                              into separate PSUM banks, co-streamed; 65th
                              v column accumulates softmax denominators)
  norm:  cxs = cxa+cxb (DVE), reciprocal_approx_fast, GPSIMD partition-
         broadcast of 1/denom, DVE mul -> ctx (fp16). No PE, no PSUM.
  out   = ctxT.T @ WoT       (PE, accumulated over head pairs)

Scheduling: the attention k-loop is ACT(exp)-bound (~1.0us/kt). The score
PSUM ring (tag "big") carries ONLY score tiles mid-loop so its double
buffering is never phase-slipped; output-projection units are batched
right after kt0/kt1 of the next k-loop (their matmuls overlap the first
exps), and PV ramps in 4 iterations late at each pair start so the
previous pair's norm (DVE-only) can release the cx PSUM banks without
stalling the PE.
"""

import sys
import types
from collections import deque

import numpy as np

# ---------------------------------------------------------------------------
# Problem constants (hardcoded; kernel.py must be self-contained)
# ---------------------------------------------------------------------------
B = 2  # batch
S = 2048  # sequence length
D = 1024  # model dim
H = 16  # heads
DK = D // H  # 64 head dim
NCORES = 8
CPB = NCORES // B  # cores per batch = 4
FH = D // CPB  # features per core = 256 (4 heads)
P = 128
KD = D // P  # 8 contraction k-tiles for projections
KT = S // P  # 16 key-token tiles
NM = FH // P  # 2 m-tiles per core = head pairs
QS = 512  # q-slice width for the attention inner loop
NQS = S // QS  # 4
VW = 66  # padded per-head width of the augmented V tile (DK + 1 + pad)
NEG_SCALE = 1.0 / np.sqrt(DK)  # folded into Wq/bq on the host


def _install_ntff_hook():
    """Recreate antenv.axon_hooks so trace=True can profile via axon."""
    if "antenv.axon_hooks" in sys.modules:
        return
    try:
        import antenv
    except ImportError:
        return
    mod = types.ModuleType("antenv.axon_hooks")
    mod._hook = None
    mod.set_axon_ntff_profile_hook = lambda h: setattr(mod, "_hook", h)
    mod.get_axon_ntff_profile_hook = lambda: mod._hook
    sys.modules["antenv.axon_hooks"] = mod
    antenv.axon_hooks = mod
    try:
        from trn_agent_boot.trn_boot import _ntff_profile_via_ctypes

        mod.set_axon_ntff_profile_hook(
            _ntff_profile_via_ctypes("/opt/axon/libaxon_pjrt.so")
        )
    except Exception:
        pass


_NC_CACHE = {}


def _build_nc():
    """Build the per-core Bass program (identical on all 8 cores)."""
    from contextlib import ExitStack

    import concourse.bass as bass  # noqa: F401
    import concourse.mybir as mybir
    import concourse.tile as tile
    from concourse import bacc

    f32 = mybir.dt.float32
    f16 = mybir.dt.float16
    AF = mybir.ActivationFunctionType

    nc = bacc.Bacc()

    xtq = nc.dram_tensor("xtq", [D, S], f16, kind="ExternalInput")
    xtk = nc.dram_tensor("xtk", [D, S], f16, kind="ExternalInput")
    xtv = nc.dram_tensor("xtv", [D, S], f16, kind="ExternalInput")
    wqt = nc.dram_tensor("wqt", [D, FH], f16, kind="ExternalInput")
    wkt = nc.dram_tensor("wkt", [D, FH], f16, kind="ExternalInput")
    wvt = nc.dram_tensor("wvt", [D, FH], f16, kind="ExternalInput")
    wot = nc.dram_tensor("wot", [FH, D], f16, kind="ExternalInput")
    bqd = nc.dram_tensor("bqd", [P, NM], f32, kind="ExternalInput")
    bkd = nc.dram_tensor("bkd", [P, NM], f32, kind="ExternalInput")
    bvr = nc.dram_tensor("bvr", [P, FH], f32, kind="ExternalInput")
    out = nc.dram_tensor("out", [S, D], f16, kind="ExternalOutput")

    with tile.TileContext(nc) as tc, ExitStack() as ctx:
        const = ctx.enter_context(tc.tile_pool(name="const", bufs=1))
        wpool = ctx.enter_context(tc.tile_pool(name="wpool", bufs=1))
        persist = ctx.enter_context(tc.tile_pool(name="persist", bufs=1))
        xpool = ctx.enter_context(tc.tile_pool(name="xpool", bufs=4))
        expool = ctx.enter_context(tc.tile_pool(name="expool", bufs=8))
        npool = ctx.enter_context(tc.tile_pool(name="npool", bufs=2))
        obpool = ctx.enter_context(tc.tile_pool(name="obpool", bufs=3))

        # --- constants ---
        bq_sb = const.tile([P, NM], f32)
        bk_sb = const.tile([P, NM], f32)
        bvr_sb = const.tile([P, FH], f32)
        nc.sync.dma_start(bq_sb, bqd[:, :])
        nc.sync.dma_start(bk_sb, bkd[:, :])
        nc.sync.dma_start(bvr_sb, bvr[:, :])

        # --- weight tiles ---
        wq_sb = wpool.tile([P, KD, FH], f16)
        wk_sb = wpool.tile([P, KD, FH], f16)
        wv_sb = wpool.tile([P, KD, FH], f16)
        wo_sb = wpool.tile([P, NM, D], f16)

        # --- persistent activations ---
        qt_sb = persist.tile([P, NM, S], f16)
        kt_sb = persist.tile([P, NM, S], f16)
        vaug_sb = persist.tile([P, KT, CPB, VW], f16)
        xv_sb = persist.tile([P, KD, S], f16)
        ctx_sb = persist.tile([P, NM, S], f16)

        # xtv streams on the scalar engine's DMA queue from t=0 so it is
        # resident by the time the V projection starts, in parallel with
        # the q/k input stream on the sync queue.
        for ko in range(KD):
            nc.scalar.dma_start(xv_sb[:, ko, :], xtv[ko * P : (ko + 1) * P, :])

        nc.vector.memset(vaug_sb, 0.0)
        nc.vector.memset(vaug_sb[:, :, :, DK : DK + 1], 1.0)

        # ------------------------------------------------------------------
        # Phase 1a: qT/kT = W @ X.T, bias via DVE tensor_scalar on the move
        # ------------------------------------------------------------------
        with tc.tile_pool(name="pp", bufs=2, space="PSUM") as pp:
            for xdram, wdram, w_sb, b_sb, dst in (
                (xtq, wqt, wq_sb, bq_sb, qt_sb),
                (xtk, wkt, wk_sb, bk_sb, kt_sb),
            ):
                # four [P, S/2] accumulators so the next projection can
                # claim its first PSUM bank after just one DVE move
                ps = [
                    pp.tile([P, S // 2], f32, tag="pp", bufs=4, name=f"ps{m}_{half}")
                    for m in range(NM)
                    for half in range(2)
                ]
                for ko in range(KD):
                    nc.sync.dma_start(
                        w_sb[:, ko, :], wdram[ko * P : (ko + 1) * P, :]
                    )
                    # split each x tile across two DMA queues for bandwidth
                    # (gpsimd's queue wakes late -- keep ko=0 fully on sync)
                    xt_t = xpool.tile([P, S], f16, tag="xt")
                    nc.sync.dma_start(
                        xt_t[:, 0 : S // 2], xdram[ko * P : (ko + 1) * P, 0 : S // 2]
                    )
                    eng = nc.sync if (xdram is xtq and ko == 0) else nc.gpsimd
                    eng.dma_start(
                        xt_t[:, S // 2 : S], xdram[ko * P : (ko + 1) * P, S // 2 : S]
                    )
                    for m in range(NM):
                        for ns in range(S // 512):
                            nc.tensor.matmul(
                                ps[2 * m + ns // 2][
                                    :, (ns % 2) * 512 : (ns % 2 + 1) * 512
                                ],
                                lhsT=w_sb[:, ko, m * P : (m + 1) * P],
                                rhs=xt_t[:, ns * 512 : (ns + 1) * 512],
                                start=(ko == 0),
                                stop=(ko == KD - 1),
                            )
                for m in range(NM):
                    for half in range(2):
                        nc.vector.tensor_scalar_add(
                            dst[:, m, half * (S // 2) : (half + 1) * (S // 2)],
                            ps[2 * m + half][:, :],
                            b_sb[:, m : m + 1],
                        )

        # ------------------------------------------------------------------
        # Phase 1b: V natural = X @ Wv.T per 128-token chunk (no transposes)
        # ------------------------------------------------------------------
        bvr4 = bvr_sb.rearrange("p (h x) -> p h x", x=DK)
        vaug4 = vaug_sb  # [P, kt, head, VW]
        for ko in range(KD):
            nc.sync.dma_start(wv_sb[:, ko, :], wvt[ko * P : (ko + 1) * P, :])
        nc.sync.dma_start(wo_sb, wot[:, :].rearrange("(m p) d -> p m d", p=P))
        with tc.tile_pool(name="vp", bufs=2, space="PSUM") as vp:
            for tcn in range(KT):
                pvn = vp.tile([P, FH], f32, tag="vn", name=f"pvn{tcn}")
                for ko in range(KD):
                    nc.tensor.matmul(
                        pvn,
                        lhsT=xv_sb[:, ko, tcn * P : (tcn + 1) * P],
                        rhs=wv_sb[:, ko, :],
                        start=(ko == 0),
                        stop=(ko == KD - 1),
                    )
                nc.vector.tensor_add(
                    vaug4[:, tcn, :, 0:DK],
                    bvr4,
                    pvn.rearrange("p (h x) -> p h x", x=DK),
                )

        # ------------------------------------------------------------------
        # Phase 2: attention + deferred norm / output projection
        # ------------------------------------------------------------------
        with (
            tc.tile_pool(name="big", bufs=2, space="PSUM") as big,
            tc.tile_pool(name="pcxa", bufs=2, space="PSUM") as pcxa,
            tc.tile_pool(name="pcxb", bufs=2, space="PSUM") as pcxb,
        ):
            pending = deque()  # DVE-only closures (norm muls)
            op_queue = deque()  # output-projection units (PE+DVE+DMA)

            def drain(n=1):
                for _ in range(n):
                    if pending:
                        pending.popleft()()

            def out_proj(mt, ns, pool, tag):
                # psum comes from a cx ring (freed by the boundary merge) so
                # the score ring's double-buffering is never disturbed
                ops = pool.tile([P, 512], f32, tag=tag, name=f"op{mt}_{ns}")
                for pair in range(NM):
                    nc.tensor.matmul(
                        ops,
                        lhsT=ctx_sb[:, pair, mt * P : (mt + 1) * P],
                        rhs=wo_sb[:, pair, ns * 512 : (ns + 1) * 512],
                        start=(pair == 0),
                        stop=(pair == NM - 1),
                    )
                ob = obpool.tile([P, 512], f16, tag="ob", name=f"ob{mt}_{ns}")
                nc.vector.tensor_copy(ob, ops)
                nc.sync.dma_start(
                    out[mt * P : (mt + 1) * P, ns * 512 : (ns + 1) * 512], ob
                )

            for qs in range(NQS):
                q0 = qs * QS
                for pair in range(NM):
                    cx = {}  # allocated lazily at the first PV emission

                    def pv(pv_kt, pv_ex, cx=cx, pair=pair, qs=qs):
                        if not cx:
                            cx["a"] = [
                                pcxa.tile(
                                    [DK + 1, QS],
                                    f32,
                                    tag="cxa",
                                    name=f"cxa{pair}_{qs}_{h}",
                                )
                                for h in range(2)
                            ]
                            cx["b"] = [
                                pcxb.tile(
                                    [DK + 1, QS],
                                    f32,
                                    tag="cxb",
                                    name=f"cxb{pair}_{qs}_{h}",
                                )
                                for h in range(2)
                            ]
                        cxa, cxb = cx["a"], cx["b"]
                        for h in range(2):
                            nc.tensor.matmul(
                                cxa[h],
                                lhsT=vaug4[0:DK, pv_kt, 2 * pair + h, 0 : DK + 1],
                                rhs=pv_ex[0:DK, h * QS : (h + 1) * QS],
                                start=(pv_kt == 0),
                                stop=(pv_kt == KT - 1),
                            )
                            nc.tensor.matmul(
                                cxb[h],
                                lhsT=vaug4[DK:P, pv_kt, 2 * pair + h, 0 : DK + 1],
                                rhs=pv_ex[DK:P, h * QS : (h + 1) * QS],
                                start=(pv_kt == 0),
                                stop=(pv_kt == KT - 1),
                            )

                    exq = deque()
                    for kt in range(KT):
                        sc = big.tile(
                            [P, 2 * QS], f32, tag="big", name=f"sc{pair}_{qs}_{kt}"
                        )
                        for h in range(2):
                            nc.tensor.matmul(
                                sc[:, h * QS : (h + 1) * QS],
                                lhsT=kt_sb[
                                    DK * h : DK * (h + 1),
                                    pair,
                                    kt * P : (kt + 1) * P,
                                ],
                                rhs=qt_sb[
                                    DK * h : DK * (h + 1), pair, q0 : q0 + QS
                                ],
                                start=True,
                                stop=True,
                            )
                        ex = expool.tile([P, 2 * QS], f16, tag="ex")
                        nc.scalar.activation(ex, sc, AF.Exp)
                        exq.append((kt, ex))
                        # out-proj units, one per kt over kt1-4, alternating
                        # cx rings (freed by the boundary merge); each
                        # overlaps one exp on ACT
                        if kt in (1, 3) and op_queue:
                            op_queue.popleft()(pcxa, "cxa")
                        elif kt in (2, 4) and op_queue:
                            op_queue.popleft()(pcxb, "cxb")
                        # PV ramps in late (cx banks release) then catches up
                        emitted = 0
                        lim = 0 if kt < 5 else 2
                        while len(exq) > 2 and emitted < lim:
                            pv(*exq.popleft())
                            emitted += 1
                        if kt >= 6:
                            drain(1)
                    while exq:
                        pv(*exq.popleft())
                    cxa, cxb = cx["a"], cx["b"]

                    # --- inline norm prep (DVE/DMA/GPSIMD only, no PE) ---
                    cxs = [
                        npool.tile(
                            [DK + 1, QS],
                            f32,
                            tag="cxs",
                            bufs=4,
                            name=f"cxs{pair}_{qs}_{h}",
                        )
                        for h in range(2)
                    ]
                    for h in range(2):
                        nc.vector.tensor_copy(cxs[h], cxa[h])
                        nc.vector.tensor_add(cxs[h], cxs[h], cxb[h])
                    s2 = npool.tile([2, QS], f32, tag="s2", name=f"s2_{pair}_{qs}")
                    nc.sync.dma_start(s2[0:1, :], cxs[0][DK : DK + 1, :])
                    nc.sync.dma_start(s2[1:2, :], cxs[1][DK : DK + 1, :])
                    rc = npool.tile([2, QS], f32, tag="rc", name=f"rc{pair}_{qs}")
                    nc.vector.reciprocal_approx_fast(rc, s2)
                    rcf = npool.tile([2, QS], f16, tag="rcf", name=f"rcf{pair}_{qs}")
                    nc.vector.tensor_copy(rcf, rc)
                    rcB = npool.tile([1, QS], f16, tag="rcB", name=f"rcB{pair}_{qs}")
                    nc.sync.dma_start(rcB, rcf[1:2, :])
                    bcs = [
                        npool.tile(
                            [DK, QS], f16, tag="bcs", bufs=4, name=f"bc{pair}_{qs}_{h}"
                        )
                        for h in range(2)
                    ]
                    nc.gpsimd.partition_broadcast(bcs[0], rcf[0:1, :])
                    nc.gpsimd.partition_broadcast(bcs[1], rcB)

                    def mul_h(p, q, h, cxs_h, bc_h):
                        nc.vector.tensor_mul(
                            ctx_sb[DK * h : DK * (h + 1), p, q * QS : (q + 1) * QS],
                            cxs_h[0:DK, :],
                            bc_h,
                        )

                    for h in range(2):
                        pending.append(
                            lambda p=pair, q=qs, hh=h, a=cxs[h], b=bcs[h]: mul_h(
                                p, q, hh, a, b
                            )
                        )

                # enqueue this qs's output projections only once both pairs'
                # norm muls have been EMITTED (the enqueue rides the pending
                # queue, which drains strictly after the muls) so out_proj
                # never reads ctx_sb ahead of its writers in program order.
                def enq_ops(q=qs):
                    for sub in range(QS // P):
                        for ns in range(D // 512):
                            op_queue.append(
                                lambda pool, tag, m=q * (QS // P) + sub, n=ns: out_proj(
                                    m, n, pool, tag
                                )
                            )

                pending.append(enq_ops)

                # tail of the qs: nothing; leftovers drain at the very end
            # final drain: leftover op units fill the PE while the last
            # pair's norm chain (DVE/GPSIMD) computes; then its own ops
            for j in range(min(4, len(op_queue))):
                if j % 2 == 0:
                    op_queue.popleft()(pcxa, "cxa")
                else:
                    op_queue.popleft()(pcxb, "cxb")
            while pending:
                drain(1)
            alt = 0
            while op_queue:
                if alt % 2 == 0:
                    op_queue.popleft()(pcxa, "cxa")
                else:
                    op_queue.popleft()(pcxb, "cxb")
                alt += 1

    nc.finalize()
    return nc


def _get_nc():
    if "nc" not in _NC_CACHE:
        _install_ntff_hook()
        _NC_CACHE["nc"] = _build_nc()
    return _NC_CACHE["nc"]


def _make_in_maps(query, key, value, Wq, bq, Wk, bk, Wv, bv, Wo):
    qn = np.asarray(query, np.float32)
    kn = np.asarray(key, np.float32)
    vn = np.asarray(value, np.float32)
    Wq = np.asarray(Wq, np.float32)
    Wk = np.asarray(Wk, np.float32)
    Wv = np.asarray(Wv, np.float32)
    Wo = np.asarray(Wo, np.float32)
    bq = np.asarray(bq, np.float32)
    bk = np.asarray(bk, np.float32)
    bv = np.asarray(bv, np.float32)

    xt = {}
    for b in range(B):
        xt[b] = (
            np.ascontiguousarray(qn[b].T).astype(np.float16),
            np.ascontiguousarray(kn[b].T).astype(np.float16),
            np.ascontiguousarray(vn[b].T).astype(np.float16),
        )

    in_maps = []
    for c in range(NCORES):
        b, hp = divmod(c, CPB)
        sl = slice(hp * FH, (hp + 1) * FH)
        in_maps.append(
            {
                "xtq": xt[b][0],
                "xtk": xt[b][1],
                "xtv": xt[b][2],
                "wqt": np.ascontiguousarray((Wq[sl, :] * NEG_SCALE).T).astype(
                    np.float16
                ),
                "wkt": np.ascontiguousarray(Wk[sl, :].T).astype(np.float16),
                "wvt": np.ascontiguousarray(Wv[sl, :].T).astype(np.float16),
                "wot": np.ascontiguousarray(Wo[:, sl].T).astype(np.float16),
                "bqd": np.ascontiguousarray(
                    (bq[sl] * NEG_SCALE).reshape(NM, P).T
                ),
                "bkd": np.ascontiguousarray(bk[sl].reshape(NM, P).T),
                "bvr": np.ascontiguousarray(np.tile(bv[sl], (P, 1))),
            }
        )
    return in_maps


def _run(inputs, trace=False):
    from concourse.bass_utils import run_bass_kernel_spmd

    nc = _get_nc()
    in_maps = _make_in_maps(
        inputs["query"],
        inputs["key"],
        inputs["value"],
        inputs["Wq"],
        inputs["bq"],
        inputs["Wk"],
        inputs["bk"],
        inputs["Wv"],
        inputs["bv"],
        inputs["Wo"],
    )
    res = run_bass_kernel_spmd(nc, in_maps, list(range(NCORES)), trace=trace)
    bo = np.asarray(inputs["bo"], np.float32)
    out = np.zeros((B, S, D), np.float32)
    for c in range(NCORES):
        out[c // CPB] += res.results[c]["out"].astype(np.float32)
    out += bo[None, None, :]
    return out, res


def kernel(**inputs) -> np.ndarray:
    out, _ = _run(inputs, trace=False)
    return out
